# revision 23
# baseline (speedup 1.0000x reference)
"""Multi-head self-attention (B=2, S=2048, D=1024, H=16, causal) on 8 TRN2 cores.

Sharding: core c handles batch b=c//4 and head-group g=c%4 (4 heads each).
Host pre-transposes x and the weight slices (and pre-casts them to bf16) so
the kernel never needs an on-chip transpose or cast:
  xT   [1024, 2048] = x[b].T                     (bf16)
  wqT/wkT/wvT [1024, 256] = W.T[:, g*256:(g+1)*256]  (bf16)
  woT  [256, 1024] = Wo[:, g*256:(g+1)*256].T    (bf16)
Host upcasts the bf16 partial outputs and sums the 4 per-group partials per
batch at the end.

On-chip dataflow per core (all matmuls bf16 with fp32 PSUM accumulation):
  qT/kT [256, 2048] (head dim on partitions), v [2048, 4*65] (with a ones
  column appended per head so the PV matmul also accumulates the softmax
  denominator in psum row 64).  Scores are computed transposed
  (scoresT[j, i]) so softmax needs no transpose at all; there is no
  max-subtraction (scores are O(+-6), exp is safe in fp32).

Causal structure per 512-query block Q: keys below the diagonal region
(chunks jc < 4Q) are computed full-width with no masking; the diagonal
512x512 region is processed per key-chunk kc with a shrinking query range
(queries kc*128..512), so only the 4 true diagonal 128x128 blocks need an
element mask and the strictly-above-diagonal blocks are never computed.

Scheduling: the TRN2 PE clock-gates to half rate (HAM K=4/8) whenever its
activity is interrupted, so the kernel keeps the tensor queue continuously
fed: the attention inner loop is software-pipelined with lookahead 2 (sc
MMs run two chunks ahead of PV MMs, hiding the scalar-engine exp), and the
projection matmul groups (q/k/v) plus the output-projection MMs of the
previous block are injected as fillers between attention chunks, so the PE
never idles at block boundaries waiting for the softmax-normalize chain
(which runs on vector+gpsimd)."""

import os
import sys

sys.path.insert(0, "/opt/trn_rl_repo")
os.environ.setdefault("MYCRO_LOCAL_CACHE", "1")

import ml_dtypes
import numpy as np

import concourse.bacc as bacc
import concourse.bass as bass
import concourse.mybir as mybir
import concourse.tile as tile
from concourse import bass_utils

# The agent image's antenv lacks axon_hooks, so bass_utils' trace path dies on
# import.  Register a shim module that lazily builds the ctypes NTFF hook.
if "antenv.axon_hooks" not in sys.modules:
    import types

    _shim = types.ModuleType("antenv.axon_hooks")
    _shim._HOOK = None

    def _set_hook(hook, _m=_shim):
        _m._HOOK = hook

    def _get_hook(_m=_shim):
        if _m._HOOK is None:
            try:
                from trn_agent_boot.trn_boot import _ntff_profile_via_ctypes

                _m._HOOK = _ntff_profile_via_ctypes("/opt/axon/libaxon_pjrt.so")
            except Exception:
                _m._HOOK = None
        return _m._HOOK

    _shim.set_axon_ntff_profile_hook = _set_hook
    _shim.get_axon_ntff_profile_hook = _get_hook
    sys.modules["antenv.axon_hooks"] = _shim

B, S, D, H = 2, 2048, 1024, 16
DK = 64                      # head dim
HC = 4                       # heads per core
GC = HC * DK                 # 256 cols per head-group
N_CORES = 8
SCALE = 1.0 / np.sqrt(DK)    # 0.125

F32 = mybir.dt.float32
BF16 = mybir.dt.bfloat16

TRACE = False
LAST_RESULTS = None


def build_bass():
    nc = bacc.Bacc("TRN2", target_bir_lowering=False, debug=False)

    xT_d = nc.dram_tensor("xT", [D, S], BF16, kind="ExternalInput")
    wqT_d = nc.dram_tensor("wqT", [D, GC], BF16, kind="ExternalInput")
    wkT_d = nc.dram_tensor("wkT", [D, GC], BF16, kind="ExternalInput")
    wvT_d = nc.dram_tensor("wvT", [D, GC], BF16, kind="ExternalInput")
    woT_d = nc.dram_tensor("woT", [GC, D], BF16, kind="ExternalInput")
    mask_d = nc.dram_tensor("mask", [128, 2, 128], F32, kind="ExternalInput")
    out_d = nc.dram_tensor("out", [S, D], BF16, kind="ExternalOutput")

    EXP = mybir.ActivationFunctionType.Exp

    with tile.TileContext(nc) as tc:
        with (
            nc.allow_low_precision(reason="bf16 matmuls, fp32 accumulation"),
            tc.tile_pool(name="const", bufs=1) as const,
            tc.tile_pool(name="work", bufs=3) as work,
            tc.tile_pool(name="apool", bufs=2) as apool,
            tc.tile_pool(name="opool", bufs=2) as opool,
            tc.tile_pool(name="rpool", bufs=2) as rpool,
            tc.tile_pool(name="psmm", bufs=3, space="PSUM") as psmm,
            tc.tile_pool(name="psout", bufs=2, space="PSUM") as psout,
        ):
            # ---- load inputs -------------------------------------------------
            # x striped over 2 DMA queues; weights on the gpsimd queue
            xT_dr = xT_d.rearrange("(o p) s -> p o s", p=128)
            dma_engines = [nc.sync, nc.scalar]
            # x split into half-tiles: A = columns 0:1024, B = 1024:2048.
            # Everything in the first ~half of the kernel consumes only A
            # halves, so the critical input window shrinks accordingly.
            xtsA = [const.tile([128, 1024], BF16, name=f"xtA{ko}") for ko in range(8)]
            xtsB = [const.tile([128, 1024], BF16, name=f"xtB{ko}") for ko in range(8)]

            def dma_x(ko, gate_on=None):
                dma_engines[ko % 2].dma_start(xtsA[ko][:], xT_dr[:, ko, 0:1024])

            def dma_xB(ko):
                dma_engines[ko % 2].dma_start(xtsB[ko][:], xT_dr[:, ko, 1024:2048])

            def x_cols(ko, c0, w):
                if c0 >= 1024:
                    return xtsB[ko][:, c0 - 1024:c0 - 1024 + w]
                assert c0 + w <= 1024
                return xtsA[ko][:, c0:c0 + w]

            wq = const.tile([128, 2, 4, GC], BF16)
            wk = const.tile([128, 2, 4, GC], BF16)
            wv = const.tile([128, 2, 4, GC], BF16)
            w_drs = {id(wq): wqT_d, id(wk): wkT_d, id(wv): wvT_d}
            for half in range(2):
                for w_sb, w_d in ((wq, wqT_d), (wk, wkT_d), (wv, wvT_d)):
                    nc.gpsimd.dma_start(
                        w_sb[:, half, :, :],
                        w_d.rearrange("(h o p) m -> p h o m", p=128, h=2)[:, half, :, :],
                    )
            maskt = const.tile([128, 2, 128], F32)
            nc.gpsimd.dma_start(maskt[:], mask_d[:])
            wo = const.tile([128, 2, D], BF16)

            ones_f = const.tile([128, 64], F32)
            nc.vector.memset(ones_f[:], 1.0)

            qts = [[const.tile([128, 1024], BF16, name=f"q{m}{s}")
                    for s in range(2)] for m in range(2)]
            kts = [[const.tile([128, 1024], BF16, name=f"k{m}{s}")
                    for s in range(2)] for m in range(2)]
            vts = []
            for io in range(16):
                vt = const.tile([128, HC * 65], BF16, name=f"v{io}")
                nc.vector.tensor_copy(
                    vt.rearrange("p (h u) -> p h u", u=65)[:, :, 64],
                    ones_f[:, 0:4],
                )
                vts.append(vt)

            # ---- projection group emitters (used as PE fillers) -------------
            def emit_qk_group(w_sb, dst, mo, sbh):
                ps = psmm.tile([128, 2, 512], F32, tag="mm", name="psqk")
                for ko in range(8):
                    for sb2 in range(2):
                        sb = 2 * sbh + sb2
                        nc.tensor.matmul(
                            ps[:, sb2, :],
                            (w_sb[:, ko // 4, ko % 4, mo * 128:(mo + 1) * 128]),
                            (x_cols(ko, sb * 512, 512)),
                            start=(ko == 0),
                            stop=(ko == 7),
                            skip_group_check=True,
                        )
                nc.vector.tensor_copy(
                    dst[mo][sbh][:], ps.rearrange("p a n -> p (a n)")
                )

            def emit_v_group(io):
                ps = psmm.tile([128, 256], F32, tag="mm", name="psv")
                for ko in range(8):
                    nc.tensor.matmul(
                        ps[:],
                        (x_cols(ko, io * 128, 128)),
                        (wv[:, ko // 4, ko % 4, :]),
                        start=(ko == 0),
                        stop=(ko == 7),
                    )
                nc.vector.tensor_copy(
                    vts[io].rearrange("p (h u) -> p h u", u=65)[:, :, 0:64],
                    ps.rearrange("p (h e) -> p h e", e=64),
                )

            # ---- attention emitters -----------------------------------------
            aTs = {}

            def emit_sc(Q, mo, ck):
                kind, idx = ck
                if kind == "full":
                    jc, qw, q0d, r0 = idx, 512, 0, 0
                else:
                    jc = 4 * Q + idx
                    qw = (4 - idx) * 128
                    q0d = idx * 128
                    r0 = idx * 128
                q0 = (Q % 2) * 512 + q0d
                sc = psmm.tile([128, 2, 512], F32, tag="mm", name="sc")
                for hp in range(2):
                    nc.tensor.matmul(
                        sc[:, hp, 0:qw],
                        (kts[mo][jc // 8][hp * 64:(hp + 1) * 64,
                               (jc % 8) * 128:(jc % 8 + 1) * 128]),
                        (qts[mo][Q // 2][hp * 64:(hp + 1) * 64, q0:q0 + qw]),
                        start=True,
                        stop=True,
                        skip_group_check=True,
                    )
                if kind == "diag":
                    nc.vector.tensor_add(
                        sc[:, :, 0:128], sc[:, :, 0:128], maskt[:]
                    )
                ex = work.tile([128, 2, 512], BF16, tag="exp", name="ex")
                nc.scalar.activation(
                    ex[:, :, 0:qw], sc[:, :, 0:qw], EXP, scale=SCALE
                )
                return (ex, jc, qw, r0, mo)

            def emit_pv(meta, out_ps, first, last):
                ex, jc, qw, r0, mo = meta
                for hp in range(2):
                    h = 2 * mo + hp
                    nc.tensor.matmul(
                        out_ps[hp][:, r0:512],
                        (vts[jc][:, h * 65:(h + 1) * 65]),
                        (ex[:, hp, 0:qw]),
                        start=first,
                        stop=last,
                        skip_group_check=True,
                    )

            def emit_norm(Q, mo, out_ps):
                aT = aTs[Q]
                dens, atts = [], []
                last = (Q, mo) == (3, 1)
                for hp in range(2):
                    den = rpool.tile([1, 512], F32, tag="den", name="den")
                    nc.vector.tensor_copy(den[:], out_ps[hp][64:65, :])
                    att = work.tile([64, 512], BF16, tag="att", name="att")
                    if last:
                        nc.scalar.copy(att[:], out_ps[hp][0:64, :])
                    else:
                        nc.vector.tensor_copy(att[:], out_ps[hp][0:64, :])
                    dens.append(den)
                    atts.append(att)
                for hp in range(2):
                    rd_f = rpool.tile([1, 512], F32, tag="rdf", name="rdf")
                    nc.vector.reciprocal_approx_fast(out=rd_f[:], in_=dens[hp][:])
                    rd = rpool.tile([1, 512], BF16, tag="rd", name="rd")
                    nc.vector.tensor_copy(rd[:], rd_f[:])
                    # broadcast 1/denom across 64 partitions on gpsimd (no
                    # PSUM, no tensor-engine involvement)
                    rdbs = work.tile([64, 512], BF16, tag="rdbs", name="rdbs")
                    nc.gpsimd.partition_broadcast(rdbs[:], rd[0:1, :])
                    nc.vector.tensor_mul(
                        aT[hp * 64:(hp + 1) * 64, mo, :],
                        atts[hp][:],
                        rdbs[:],
                    )

            def emit_oproj_so(Q, so):
                aT = aTs[Q]
                osb = opool.tile([128, D], BF16, tag="osb", name="osb")
                po = psmm.tile([128, 2, 512], F32, tag="mm", name="po")
                for co in range(2):
                    for nt in range(2):
                        nc.tensor.matmul(
                            po[:, nt, :],
                            (aT[:, co, so * 128:(so + 1) * 128]),
                            (wo[:, co, nt * 512:(nt + 1) * 512]),
                            start=(co == 0),
                            stop=(co == 1),
                            skip_group_check=True,
                        )
                nc.vector.tensor_copy(osb[:], po.rearrange("p a n -> p (a n)"))
                dma_engines[so % 2].dma_start(
                    out_d.rearrange("(a p) n -> p a n", p=128)[:, Q * 4 + so, :],
                    osb[:],
                )

            # ---- schedule ----------------------------------------------------
            # PE fillers injected between attention chunk-steps, placed so
            # every projection group lands before its first consumer.
            fillers = {
                (0, 0): {1: [lambda: emit_qk_group(wq, qts, 1, 0)],
                         2: [lambda: emit_qk_group(wk, kts, 1, 0)]},
                (0, 1): {1: [lambda: emit_v_group(4)],
                         2: [lambda: emit_v_group(5)],
                         3: [lambda: emit_v_group(6)]},
                (1, 0): {1: [lambda: emit_v_group(7)],
                         3: [lambda: emit_qk_group(wq, qts, 0, 1)],
                         5: [lambda: emit_qk_group(wk, kts, 0, 1)]},
                (1, 1): {1: [lambda: emit_v_group(8)],
                         3: [lambda: emit_v_group(9)],
                         5: [lambda: emit_qk_group(wq, qts, 1, 1)]},
                (2, 0): {1: [lambda: emit_v_group(10)],
                         3: [lambda: emit_v_group(11)],
                         5: [lambda: emit_qk_group(wk, kts, 1, 1)]},
                (2, 1): {1: [lambda: emit_v_group(12)],
                         3: [lambda: emit_v_group(13)],
                         5: [lambda: emit_v_group(14)]},
                (3, 0): {1: [lambda: emit_v_group(15)]},
                (3, 1): {},
            }
            # previous block's output projection, spread one `so` at a time
            for Q in range(3):
                blk = fillers[(Q + 1, 0)]
                nsteps = 4 * (Q + 1) + 4
                steps = [2, 4, 6, 8] if nsteps >= 10 else [2, 3, 4, 5]
                for so in range(4):
                    blk.setdefault(steps[so], []).append(
                        lambda Q=Q, so=so: emit_oproj_so(Q, so)
                    )

            # prologue: q00+k00+v0+v1 interleaved per contraction chunk, with
            # the x DMAs issued two chunks ahead so the PE computes on chunk
            # ko while ko+2 streams in (fine-grained DMA waits)
            dma_x(0)
            dma_x(1)
            ps_q = psmm.tile([128, 2, 512], F32, tag="mm", name="ps_q")
            ps_k = psmm.tile([128, 2, 512], F32, tag="mm", name="ps_k")
            ps_v0 = psout.tile([128, 256], F32, tag="out", name="ps_v0")
            ps_v1 = psout.tile([128, 256], F32, tag="out", name="ps_v1")
            for ko in range(8):
                if ko + 2 < 8:
                    dma_x(ko + 2, gate_on=ko)
                for w_sb, ps in ((wq, ps_q), (wk, ps_k)):
                    for sb in range(2):
                        nc.tensor.matmul(
                            ps[:, sb, :],
                            (w_sb[:, ko // 4, ko % 4, 0:128]),
                            (xtsA[ko][:, sb * 512:(sb + 1) * 512]),
                            start=(ko == 0),
                            stop=(ko == 7),
                            skip_group_check=True,
                        )
                for io, ps in ((0, ps_v0), (1, ps_v1)):
                    nc.tensor.matmul(
                        ps[:],
                        (xtsA[ko][:, io * 128:(io + 1) * 128]),
                        (wv[:, ko // 4, ko % 4, :]),
                        start=(ko == 0),
                        stop=(ko == 7),
                    )
            nc.vector.tensor_copy(qts[0][0][:], ps_q.rearrange("p a n -> p (a n)"))
            nc.vector.tensor_copy(kts[0][0][:], ps_k.rearrange("p a n -> p (a n)"))
            for io, ps in ((0, ps_v0), (1, ps_v1)):
                nc.vector.tensor_copy(
                    vts[io].rearrange("p (h u) -> p h u", u=65)[:, :, 0:64],
                    ps.rearrange("p (h e) -> p h e", e=64),
                )
            for ko in range(8):
                dma_xB(ko)
            nc.gpsimd.dma_start(wo[:], woT_d.rearrange("(o p) n -> p o n", p=128))
            emit_v_group(2)
            emit_v_group(3)

            LOOK = 2
            for Q in range(4):
                aTs[Q] = apool.tile([128, 2, 512], BF16, tag="aT", name=f"aT{Q}")
                for mo in range(2):
                    chunks = ([("full", jc) for jc in range(4 * Q)]
                              + [("diag", kc) for kc in range(4)])
                    n = len(chunks)
                    out_ps = [
                        psout.tile([65, 512], F32, tag="out", name=f"out_ps{_h}")
                        for _h in range(2)
                    ]
                    fill = fillers[(Q, mo)]
                    metas = []
                    for i, ck in enumerate(chunks):
                        metas.append(emit_sc(Q, mo, ck))
                        for f in fill.get(i, []):
                            f()
                        if i >= LOOK:
                            j = i - LOOK
                            emit_pv(metas[j], out_ps, j == 0, j == n - 1)
                            metas[j] = None
                    for j in range(max(0, n - LOOK), n):
                        emit_pv(metas[j], out_ps, j == 0, j == n - 1)
                    emit_norm(Q, mo, out_ps)
            for so in range(4):
                emit_oproj_so(3, so)

    nc.compile()
    return nc


_NC = None


def _get_nc():
    global _NC
    if _NC is None:
        _NC = build_bass()
    return _NC


def _causal_mask():
    j = np.arange(128)[:, None, None]
    i = np.arange(128)[None, None, :]
    m = np.where(j <= i, 0.0, -1e9).astype(np.float32)
    return np.ascontiguousarray(np.broadcast_to(m, (128, 2, 128)))


def kernel(in_features, Wq, Wk, Wv, Wo):
    global LAST_RESULTS
    nc = _get_nc()

    bf = ml_dtypes.bfloat16
    x = np.asarray(in_features, np.float32)
    Wq = np.asarray(Wq, np.float32)
    Wk = np.asarray(Wk, np.float32)
    Wv = np.asarray(Wv, np.float32)
    Wo = np.asarray(Wo, np.float32)
    mask = _causal_mask()

    in_maps = []
    for c in range(N_CORES):
        b, g = divmod(c, 4)
        cols = slice(g * GC, (g + 1) * GC)
        in_maps.append({
            "xT": np.ascontiguousarray(x[b].T).astype(bf),
            "wqT": np.ascontiguousarray(Wq.T[:, cols]).astype(bf),
            "wkT": np.ascontiguousarray(Wk.T[:, cols]).astype(bf),
            "wvT": np.ascontiguousarray(Wv.T[:, cols]).astype(bf),
            "woT": np.ascontiguousarray(Wo[:, cols].T).astype(bf),
            "mask": mask,
        })

    res = bass_utils.run_bass_kernel_spmd(
        nc, in_maps, core_ids=list(range(N_CORES)), trace=TRACE,
    )
    LAST_RESULTS = res
    parts = [res.results[c]["out"].astype(np.float32) for c in range(N_CORES)]
    out = np.stack([
        parts[4 * b] + parts[4 * b + 1] + parts[4 * b + 2] + parts[4 * b + 3]
        for b in range(B)
    ]).astype(np.float32)
    return out
